# revision 14
# baseline (speedup 1.0000x reference)
"""MoE FFN (top-2 of 8 experts, d_model=1024, d_hid=4096) on 8 TRN2 NeuronCores.

Strategy (expert-parallel, per the sharding hint):
  - Router (tiny [N,1024]@[1024,8] matmul + softmax + top-2) is computed on
    the host; it is 0.006% of the FLOPs and produces the data-dependent
    dispatch ("all-to-all") pattern.
  - Each of the 8 cores owns one expert: it receives only the tokens routed
    to its expert (gathered, transposed, padded to capacity C, cast bf16)
    plus its expert's w1/w2 (bf16) and b1 (f32).
  - Device per core:  hT = gelu(w1^T @ xgT + b1)   [H=4096, C]   (bf16)
                      out = hT^T @ w2              [C, D=1024]   (f32)
    Gelu+bias is fused into the PSUM->SBUF eviction on the scalar engine.
  - Host combine: out_full[token] += top_w * out_core[row] (+ gates @ b2).

The matmuls are bf16 (rel-err ~1e-3 vs the f32 reference, well inside the
2e-2 gate); accumulation is f32 in PSUM.
"""

import numpy as np
import ml_dtypes

import concourse.bass as bass
import concourse.mybir as mybir
import concourse.tile as tile
from concourse import bacc
from concourse.bass_utils import run_bass_kernel_spmd
from concourse.kernels.tile_matmul import (
    composable_matmul_tile_kernel,
    dma_from_dram_kxm,
    dma_from_dram_kxn,
    dma_to_dram_mxn,
    k_pool_min_bufs,
    matmul_tile_kernel,
    scalar_copyback,
)

D_MODEL, D_HID, N_EXPERTS, TOP_K = 1024, 4096, 8, 2
N_CORES = 8
P = 128

BF16 = mybir.dt.bfloat16
F32 = mybir.dt.float32

_program_cache: dict[int, object] = {}


def _pick_n_tile(C: int) -> int:
    """Largest tile <= 512 that divides C exactly (C is a multiple of 128),
    so phase 1 never computes padded garbage columns."""
    for cand in (512, 384, 256, 128):
        if C % cand == 0:
            return cand
    return 128


def _build_program(
    C: int,
    repeat: int = 1,
    vec_copyback: bool = True,
    p2_k_tile: int = 512,
    p1_k_tile: int = 512,
    psum_bufs: int = 2,
):
    """SPMD Bass program: one expert's FFN over C (padded) routed tokens.

    Phase 1:  hT[H, C] = gelu(w1^T @ xgT + b1)   -- hT resident in SBUF, bf16
    Phase 2:  out[C, D] = hT^T @ w2              -- f32 out

    repeat > 1 wraps the body in an on-device For_i loop (benchmarking:
    one NEFF execution runs the FFN `repeat` times back-to-back).
    """
    from concourse.bass import ds, ts
    from concourse.kernels.tile_matmul import ShapeInfo

    nc = bacc.Bacc(
        "TRN2",
        target_bir_lowering=False,
        debug=False,
        num_devices=N_CORES,
    )
    xgT = nc.dram_tensor("xgT", [D_MODEL, C], BF16, kind="ExternalInput").ap()
    w1 = nc.dram_tensor("w1", [D_MODEL, D_HID], BF16, kind="ExternalInput").ap()
    w2 = nc.dram_tensor("w2", [D_HID, D_MODEL], BF16, kind="ExternalInput").ap()
    b1 = nc.dram_tensor("b1", [P, D_HID // P], F32, kind="ExternalInput").ap()
    out = nc.dram_tensor("out", [C, D_MODEL], F32, kind="ExternalOutput").ap()

    HO = D_HID // P  # 32 h-outer blocks
    N_TILE_1 = _pick_n_tile(C)

    with tile.TileContext(nc) as tc:
        with (
            tc.tile_pool(name="const", bufs=1) as const_pool,
            tc.tile_pool(name="ht_res", bufs=1) as ht_pool,
            tc.tile_pool(
                name="p1_kxm",
                bufs=2 * k_pool_min_bufs(w1, max_tile_size=p1_k_tile),
            ) as p1_kxm_pool,
            tc.tile_pool(name="xg_res", bufs=1) as xg_pool,
            tc.tile_pool(
                name="p2_kxn", bufs=k_pool_min_bufs(w2, max_tile_size=p2_k_tile) + 3
            ) as p2_kxn_pool,
        ):
            b1_sb = const_pool.tile([P, D_HID // P], F32)
            nc.sync.dma_start(b1_sb[:], b1[:])

            # hT resident in SBUF: [128, 32, C] bf16 (~72KB/partition @ C=1152)
            hT_sb = ht_pool.tile([P, HO, C], BF16)
            # xgT resident in SBUF: [128, 8, C] bf16 (~18KB/partition)
            DO = D_MODEL // P
            xg_sb = xg_pool.tile([P, DO, C], BF16)
            xgT_r = xgT.rearrange("(po pi) f -> pi po f", pi=P)

            def gelu_bias_reducer(nc_, psum, sbuf, md):
                blk = md.m_tile_idx * md.m_subtiles + md.m_subtile_idx
                nc_.scalar.activation(
                    sbuf,
                    psum,
                    mybir.ActivationFunctionType.Gelu,
                    bias=b1_sb[:, blk : blk + 1],
                )

            def p1_mxn_producer(nc_, md):
                return hT_sb[
                    :,
                    ts(md.m_tile_idx, md.m_subtiles),
                    ds(md.n_tile_idx * md.n_tile, md.n_tile),
                ]

            def p2_kxm_producer(nc_, md):
                return hT_sb[
                    :,
                    ts(md.k_tile_idx, md.k_subtiles),
                    ds(md.m_tile_idx * md.m_tile, md.m_tile),
                ]

            def p1_kxn_producer(nc_, md):
                return xg_sb[
                    :,
                    ts(md.k_tile_idx, md.k_subtiles),
                    ds(md.n_tile_idx * md.n_tile, md.n_tile),
                ]

            def vector_copyback(nc_, psum, sbuf, md):
                nc_.vector.tensor_copy(out=sbuf, in_=psum)

            def body():
                # Load all routed tokens into resident SBUF (8 x 288KB DMAs),
                # spread across engine queues so SWDGE prep (~1us each)
                # doesn't serialize ahead of the first matmul.
                dma_engines = [nc.gpsimd, nc.scalar]
                for j in range(DO):
                    dma_engines[j % len(dma_engines)].dma_start(
                        xg_sb[:, j], xgT_r[:, j]
                    )

                # Phase 1: hT = gelu(w1^T @ xgT + b1), written into hT_sb
                kxm_producer, kxm_shape = dma_from_dram_kxm(p1_kxm_pool, w1)
                composable_matmul_tile_kernel(
                    tc,
                    kxm_shape=kxm_shape,
                    kxn_shape=ShapeInfo(pdims=((P, DO),), fdims=(C,)),
                    output_type=None,
                    kxm_producer=kxm_producer,
                    kxn_producer=p1_kxn_producer,
                    mxn_consumer=lambda nc_, sbuf, md: None,
                    mxn_subtile_reducer=gelu_bias_reducer,
                    mxn_subtile_producer=p1_mxn_producer,
                    MAX_TILE_SIZE=N_TILE_1,
                    MAX_K_TILE_SIZE=p1_k_tile,
                    psum_n_bufs=psum_bufs,
                )

                # Phase 2: out = hT^T @ w2 (kxm served from resident SBUF)
                kxn2_producer, kxn2_shape = dma_from_dram_kxn(p2_kxn_pool, w2)
                composable_matmul_tile_kernel(
                    tc,
                    kxm_shape=ShapeInfo(pdims=((P, HO),), fdims=(C,)),
                    kxn_shape=kxn2_shape,
                    output_type=F32,
                    kxm_producer=p2_kxm_producer,
                    kxn_producer=kxn2_producer,
                    mxn_consumer=dma_to_dram_mxn(out),
                    mxn_subtile_reducer=(
                        vector_copyback if vec_copyback else scalar_copyback()
                    ),
                    MAX_K_TILE_SIZE=p2_k_tile,
                    psum_n_bufs=psum_bufs,
                )

            if repeat > 1:
                with tc.For_i(0, repeat, 1):
                    body()
            else:
                body()

    nc.compile()
    return nc


def _route(x, gate_w):
    """Host router: softmax + top-2 + renormalize. Returns dispatch lists."""
    xf = np.ascontiguousarray(np.asarray(x, dtype=np.float32)).reshape(-1, D_MODEL)
    n_tok = xf.shape[0]
    gw = np.asarray(gate_w, dtype=np.float32)
    logits = xf @ gw.T  # [N, E]
    m = logits.max(axis=-1, keepdims=True)
    e = np.exp(logits - m, dtype=np.float32)
    scores = e / e.sum(axis=-1, keepdims=True)
    # top-2 (softmax is monotone in logits; use scores to mirror the reference)
    top_i = np.argpartition(-scores, TOP_K - 1, axis=-1)[:, :TOP_K]  # [N, K]
    top_w = np.take_along_axis(scores, top_i, axis=-1)
    top_w = top_w / top_w.sum(axis=-1, keepdims=True)
    idx_per_e, w_per_e = [], []
    for ex in range(N_EXPERTS):
        tok, slot = np.nonzero(top_i == ex)
        idx_per_e.append(tok)
        w_per_e.append(top_w[tok, slot])
    return xf, n_tok, scores, idx_per_e, w_per_e


def _run_device(x, gate_w, w1, b1, w2, b2, trace=False, trace_kwargs=None):
    xf, n_tok, _scores, idx_per_e, w_per_e = _route(x, gate_w)

    max_count = max(len(ix) for ix in idx_per_e)
    C = max(P, ((max_count + P - 1) // P) * P)

    if C not in _program_cache:
        _program_cache[C] = _build_program(C)
    nc = _program_cache[C]

    w1 = np.asarray(w1, dtype=np.float32)
    w2 = np.asarray(w2, dtype=np.float32)
    b1 = np.asarray(b1, dtype=np.float32).reshape(N_EXPERTS, D_HID)
    b2 = np.asarray(b2, dtype=np.float32).reshape(N_EXPERTS, D_MODEL)

    in_maps = []
    for ex in range(N_CORES):
        ix = idx_per_e[ex]
        xgT = np.zeros((D_MODEL, C), dtype=ml_dtypes.bfloat16)
        xgT[:, : len(ix)] = xf[ix].T.astype(ml_dtypes.bfloat16)
        in_maps.append(
            {
                "xgT": xgT,
                "w1": w1[ex].astype(ml_dtypes.bfloat16),
                "w2": w2[ex].astype(ml_dtypes.bfloat16),
                # b1[e] laid out [P, H/P] with partition fastest: b1[mo*128+p] -> [p, mo]
                "b1": np.ascontiguousarray(
                    b1[ex].reshape(D_HID // P, P).T
                ),
            }
        )

    kw = {}
    if trace:
        kw["trace"] = True
        if trace_kwargs:
            kw["trace_kwargs"] = trace_kwargs
    res = run_bass_kernel_spmd(nc, in_maps, core_ids=list(range(N_CORES)), **kw)

    out_full = np.zeros((n_tok, D_MODEL), dtype=np.float32)
    for ex in range(N_CORES):
        ix = idx_per_e[ex]
        dev_out = np.asarray(res.results[ex]["out"], dtype=np.float32)
        out_full[ix] += w_per_e[ex][:, None] * dev_out[: len(ix)]
    # b2 term: sum_e gate[n,e] * b2[e]  (gates of unselected experts are zero)
    gates = np.zeros((n_tok, N_EXPERTS), dtype=np.float32)
    for ex in range(N_EXPERTS):
        gates[idx_per_e[ex], ex] = w_per_e[ex]
    out_full += gates @ b2
    return out_full, res


def kernel(x, gate_w, w1, b1, w2, b2):
    out_full, _res = _run_device(x, gate_w, w1, b1, w2, b2, trace=False)
    B, T, _ = np.asarray(x).shape
    return out_full.reshape(B, T, D_MODEL)
